# revision 12
# baseline (speedup 1.0000x reference)
"""Distributed Trainium2 kernel for the ABS-MAE partial-label loss.

Math: for p = softmax(outputs, axis=1) and eye the CxC identity,
    sum_k |p[n,k] - eye[j,k]| = (1 - p[n,j]) + |p[n,j] - 1| = 2 - 2*p[n,j]
so with conf = label_confidence[index] (rows of conf sum to 1),
    loss_mean = (1/N) * sum_n sum_j conf[n,j] * (2 - 2*p[n,j])
              = 2 - (2/N) * sum_n <p[n], conf[n]>.

Sharding (8 cores): label_confidence is row-sharded (6250 rows/core) and
the batch is sharded by ownership — core c handles exactly the batch items
whose index falls in its table shard (padded to K slots), so no cross-core
row movement is needed.

Device layout: each logical row is split across Q = 128/K partitions so all
128 SBUF partitions stay busy (ACT/DVE time scales with free-dim length,
not partition count).  The conf gather is one indirect DMA over the table
viewed as [ROWS*Q, C/Q] with host-expanded per-quarter indices.  Per row:
e = exp(x) (logits are N(0,1), no max-shift needed) with per-partition sums
accumulated by the activation; a one-hot selection matmul group-sums the
partition sums back to K logical rows for the softmax normalizer; rowdot =
<e, conf> per partition, group-summed the same way; the final partial is a
[K]-dot on the TensorEngine of rowdot against mask/sumexp (mask pre-scaled
by -2/N on host; 0 on pad slots).  Each core outputs
    out_c = 2/8 - (2/N) * sum_own <p, conf>,
and unsharding sums the 8 partials:
    sum_c out_c = 2 - (2/N) * sum_n <p_n, conf_n> = loss_mean.
"""

import sys
import types

import numpy as np

try:  # pragma: no cover
    import antenv.axon_hooks  # noqa: F401
except Exception:
    # bass_utils imports antenv.axon_hooks when profiling is requested
    # (BASS_TRACE=1) under axon; some containers ship an antenv stub
    # without it. Provide a no-op registry so tracing degrades gracefully
    # instead of crashing the run.
    _m = types.ModuleType("antenv.axon_hooks")
    _hook = [None]
    _m.set_axon_ntff_profile_hook = lambda h: _hook.__setitem__(0, h)
    _m.get_axon_ntff_profile_hook = lambda: _hook[0]
    sys.modules["antenv.axon_hooks"] = _m

import concourse.bass as bass
import concourse.bacc as bacc
import concourse.mybir as mybir
import concourse.tile as tile
from concourse.bass_utils import run_bass_kernel_spmd

N = 128          # batch
C = 1000         # classes
NUM_DATA = 50000 # table rows
CORES = 8
ROWS = NUM_DATA // CORES  # 6250 per-core table shard
P = 128          # SBUF partitions

_nc_cache = {}
LAST_RESULTS = None  # BassKernelResults from the most recent run (for test harness)


def _build(K):
    Q = P // K        # partitions per logical row
    CQ = C // Q       # columns per partition
    W = CQ + 1 + K    # packed input: [x_q | mask | sel]
    f32 = mybir.dt.float32
    i32 = mybir.dt.int32
    EXP = mybir.ActivationFunctionType.Exp
    CPY = mybir.ActivationFunctionType.Copy
    nc = bacc.Bacc(
        "TRN2", target_bir_lowering=False, debug=False, num_devices=CORES
    )

    xall_ext = nc.dram_tensor("xall", [P, W], f32, kind="ExternalInput")
    t_ext = nc.dram_tensor("table", [ROWS * Q, CQ], f32, kind="ExternalInput")
    gidx_ext = nc.dram_tensor("gidx", [P, 1], i32, kind="ExternalInput")
    out_ext = nc.dram_tensor("out", [1, 1], f32, kind="ExternalOutput")

    with tile.TileContext(nc) as tc:
        with (
            tc.tile_pool(name="sbuf", bufs=1) as sb,
            tc.tile_pool(name="psum", bufs=1, space="PSUM") as ps,
        ):
            # scratch: col0 = per-partition sumexp, col1 = per-partition rowdot,
            # col2 = final out, col3 = warm-exp sink
            sml = sb.tile([P, 4], f32)

            # dummy activation: pull the ACT exp table in while DMAs fly
            nc.vector.memset(sml[0:1, 2:3], 0.0)
            nc.scalar.activation(out=sml[0:1, 3:4], in_=sml[0:1, 2:3], func=EXP)

            # ---- loads (gather index first: it gates the indirect DMA) ----
            gidx = sb.tile([P, 1], i32)
            nc.sync.dma_start(out=gidx[:], in_=gidx_ext[:])
            xall = sb.tile([P, W], f32)
            nc.sync.dma_start(out=xall[:], in_=xall_ext[:])

            # ---- gather conf quarter-rows for the owned batch items ----
            conf = sb.tile([P, CQ], f32)
            nc.gpsimd.indirect_dma_start(
                out=conf[:],
                out_offset=None,
                in_=t_ext[:],
                in_offset=bass.IndirectOffsetOnAxis(ap=gidx[:, :1], axis=0),
            )

            # ---- e = exp(x) with per-partition sums ----
            e = sb.tile([P, CQ], f32)
            nc.scalar.activation(
                out=e[:],
                in_=xall[:, 0:CQ],
                func=EXP,
                bias=0.0,
                scale=1.0,
                accum_out=sml[:, 0:1],
            )

            # ---- sumexp per logical row via selection matmul; w = mask/sumexp ----
            sel = xall[:, CQ + 1 : CQ + 1 + K]
            s32p = ps.tile([K, 1], f32)
            nc.tensor.matmul(
                out=s32p[:], lhsT=sel, rhs=sml[:, 0:1], start=True, stop=True
            )
            rw = sb.tile([K, 2], f32)  # col0 = 1/sumexp, col1 = w
            nc.vector.reciprocal(out=rw[:, 0:1], in_=s32p[:])
            nc.vector.tensor_mul(rw[:, 1:2], xall[0:K, CQ : CQ + 1], rw[:, 0:1])

            # ---- rowdot per partition (prod in place), group-sum, total ----
            nc.vector.tensor_mul(e[:], e[:], conf[:])
            nc.vector.reduce_sum(
                out=sml[:, 1:2], in_=e[:], axis=mybir.AxisListType.X
            )
            r32p = ps.tile([K, 1], f32)
            nc.tensor.matmul(
                out=r32p[:], lhsT=sel, rhs=sml[:, 1:2], start=True, stop=True
            )
            r32 = sb.tile([K, 1], f32)
            nc.vector.tensor_copy(out=r32[:], in_=r32p[:])
            acc = ps.tile([1, 1], f32)
            nc.tensor.matmul(
                out=acc[:], lhsT=r32[:], rhs=rw[:, 1:2], start=True, stop=True
            )

            # ---- out_c = partial + 2/CORES ----
            nc.scalar.activation(
                out=sml[0:1, 2:3], in_=acc[:], func=CPY, bias=2.0 / CORES, scale=1.0
            )
            nc.sync.dma_start(out=out_ext[:], in_=sml[0:1, 2:3])

    nc.compile()
    return nc


def _get_nc(K):
    if K not in _nc_cache:
        _nc_cache[K] = _build(K)
    return _nc_cache[K]


def kernel(outputs, label_confidence, index):
    global LAST_RESULTS
    outputs = np.ascontiguousarray(np.asarray(outputs, dtype=np.float32))
    label_confidence = np.ascontiguousarray(
        np.asarray(label_confidence, dtype=np.float32)
    )
    idx = np.asarray(index).astype(np.int64).reshape(N)

    owner = idx // ROWS
    counts = np.bincount(owner, minlength=CORES)
    K = 32
    while K < int(counts.max()):
        K *= 2
    Q = P // K
    CQ = C // Q
    W = CQ + 1 + K
    nc = _get_nc(K)

    sel = np.zeros((P, K), dtype=np.float32)
    sel[np.arange(P), np.arange(P) // Q] = 1.0

    in_maps = []
    for c in range(CORES):
        rows = np.nonzero(owner == c)[0]
        n_own = len(rows)
        rows_p = np.concatenate([rows, np.zeros(K - n_own, dtype=rows.dtype)])
        # per-quarter gather indices into the [ROWS*Q, CQ] table view
        g = (idx[rows_p] - c * ROWS).astype(np.int64)
        g[n_own:] = 0
        gidx = (g[:, None] * Q + np.arange(Q)[None, :]).astype(np.int32).reshape(P, 1)
        mask = np.full(K, -2.0 / N, dtype=np.float32)
        mask[n_own:] = 0.0

        xall = np.zeros((P, W), dtype=np.float32)
        xall[:, 0:CQ] = outputs[rows_p].reshape(P, CQ)
        xall[0:K, CQ] = mask
        xall[:, CQ + 1 :] = sel
        in_maps.append(
            {
                "xall": xall,
                "table": label_confidence[c * ROWS : (c + 1) * ROWS].reshape(
                    ROWS * Q, CQ
                ),
                "gidx": gidx,
            }
        )
    LAST_RESULTS = run_bass_kernel_spmd(nc, in_maps, core_ids=list(range(CORES)))
    total = np.float32(0.0)
    for c in range(CORES):
        total += np.float32(LAST_RESULTS.results[c]["out"][0, 0])
    return np.asarray(total, dtype=np.float32).reshape(())
